# revision 24
# baseline (speedup 1.0000x reference)
"""Trainium2 Bass kernel for GQA attention with QK-RMSNorm, RoPE and a
bidirectional-prefix + causal mask (sparse_attention problem).

Reference computation (fp32):
  xq = x @ wq.T; xk = x @ wk.T; xv = x @ wv.T   (per-head RMSNorm on q,k)
  rope(q), rope(k); repeat kv heads 8x
  scores = q k^T / sqrt(128); mask = causal OR (i<p & j<p)
  out = softmax(scores) @ v;  y = out @ wo.T

Sharding: 8 cores = 2 batches x 4 head-groups (4 query heads each, sharing
one KV head).  Each core computes a partial y^T (its 4 heads' contribution);
the host sums the 4 partials per batch and transposes back.

All matmul operands are bf16 (fp32 PSUM accumulation); host pre-converts and
pre-tiles every input so DMAs land in their final SBUF layout with no
on-device dtype staging.  Row sums use an all-ones [128,128] stationary so
the softmax denominator lands pre-broadcast across partitions.  Scores /
row-sums / AV matmuls are narrowed to the exact masked extent per key block;
the causal diagonal is applied post-exp with a 0/1 triangular mask multiply.
Emission interleaves scores(u) with consume(u-1) and WO so the PE never
stalls on the scalar engine's exp.

TRN2 ISA allows ONE sync-wait per instruction and walrus does not split
multi-wait instructions, so `_legalize_waits` rewrites the emitted BIR,
moving excess waits onto preceding same-engine NoOps.
"""
import math
import numpy as np
from contextlib import ExitStack

import bass_rust
import concourse.bass as bass
import concourse.mybir as mybir
import concourse.tile as tile
from concourse.bass_utils import run_bass_kernel_spmd

F32 = mybir.dt.float32
BF = mybir.dt.bfloat16
AF = mybir.ActivationFunctionType

B, S, D = 2, 2048, 2048
NH, KVH, HD = 16, 2, 128
HPC = 4                      # query heads per core
N_CORES = 8
EPS = 1e-6
SOFT_SCALE = 1.0 / math.sqrt(HD)

SB = S // 128                # 16 token blocks
DB = D // 128                # 16 contraction blocks
NG = SB // 4                 # 4 query groups of 512 tokens

_lgw_counter = [0]


def _legalize_waits(nc, cap=1):
    """Move all-but-`cap` sync waits of every instruction onto preceding
    same-engine NoOps (TRN2 EVENTS block has a single wait slot)."""
    for fn in nc.m.functions:
        for blk in fn.blocks:
            out = []
            changed = False
            for inst in blk.instructions:
                si = inst.sync_info
                waits = list(si.on_wait) if si is not None and si.on_wait else []
                if len(waits) > cap:
                    changed = True
                    move, keep = waits[:-cap], waits[-cap:]
                    for w in move:
                        n = bass_rust.InstNoOp(name=f"LGW-{_lgw_counter[0]}")
                        _lgw_counter[0] += 1
                        n.engine = inst.engine
                        n.sync_info = mybir.SyncInfo(on_wait=[w], on_update=[])
                        out.append(n)
                    inst.sync_info = mybir.SyncInfo(
                        on_wait=keep, on_update=list(si.on_update or []))
                out.append(inst)
            if changed:
                blk.instructions = out
    return nc


def _eblk(rb, p):
    """Key extent (in 128-blocks) attended by query row-block rb."""
    hi = (rb + 1) * 128
    return (p if hi <= p else hi) // 128


def build_core_kernel(p, legalize=True):
    """One SPMD program; per-core behavior differs only via input data."""
    nc = bass.Bass()

    xh = nc.dram_tensor("xh", [128, DB, S], BF, kind="ExternalInput")
    wq = nc.dram_tensor("wq", [128, DB, HPC * HD], BF, kind="ExternalInput")
    wkv = nc.dram_tensor("wkv", [128, DB, 2 * HD], BF, kind="ExternalInput")
    wo = nc.dram_tensor("wo", [128, HPC, D], BF, kind="ExternalInput")
    cosq = nc.dram_tensor("cosq", [128, SB, HD], BF, kind="ExternalInput")
    sinq = nc.dram_tensor("sinq", [128, SB, HD], BF, kind="ExternalInput")
    cosk = nc.dram_tensor("cosk", [128, SB, HD], BF, kind="ExternalInput")
    sink = nc.dram_tensor("sink", [128, SB, HD], BF, kind="ExternalInput")
    identd = nc.dram_tensor("identd", [128, 128], BF, kind="ExternalInput")
    trid = nc.dram_tensor("trid", [128, 128], BF, kind="ExternalInput")
    onesd = nc.dram_tensor("onesd", [128, 128], BF, kind="ExternalInput")
    yT = nc.dram_tensor("yT", [D, S], BF, kind="ExternalOutput")

    eblks = [_eblk(rb, p) for rb in range(SB)]

    with tile.TileContext(nc) as tc, ExitStack() as octx:
        # Const/weight loads go on the Activation HWDGE queue so they overlap
        # the x-tile loads on the SP queue; ordered by first use (wq -> wkv
        # -> rope tables -> identity; wo/tri/ones only matter in phase 2).
        const = octx.enter_context(tc.tile_pool(name="const", bufs=1))
        # Separate tiles per weight chunk -> the first projection matmul only
        # waits on the first 4-block chunk, not the whole weight load.
        wq_sbs = []
        for c4 in range(4):
            wq_c = const.tile([128, 4, HPC * HD], BF, name=f"wq_c{c4}")
            nc.scalar.dma_start(out=wq_c, in_=wq[:, c4 * 4:(c4 + 1) * 4, :])
            wq_sbs.append(wq_c)
        wkv_sbs = []
        for c2 in range(2):
            wkv_c = const.tile([128, 8, 2 * HD], BF, name=f"wkv_c{c2}")
            nc.scalar.dma_start(out=wkv_c, in_=wkv[:, c2 * 8:(c2 + 1) * 8, :])
            wkv_sbs.append(wkv_c)
        # Rope tables ride the SP queue (behind x0/x1, ahead of x2+) so the
        # Act queue reaches wq/wkv sooner.
        cq_sb = const.tile([128, SB, HD], BF)
        sq_sb = const.tile([128, SB, HD], BF)
        ck_sb = const.tile([128, SB, HD], BF)
        sk_sb = const.tile([128, SB, HD], BF)
        ident = const.tile([128, 128], BF)
        nc.scalar.dma_start(out=ident, in_=identd[:, :])
        wo_sb = const.tile([128, HPC, D], BF)
        nc.scalar.dma_start(out=wo_sb, in_=wo[:, :, :])
        tri = const.tile([128, 128], BF)
        nc.scalar.dma_start(out=tri, in_=trid[:, :])
        ones_m = const.tile([128, 128], BF)
        nc.scalar.dma_start(out=ones_m, in_=onesd[:, :])
        eps_t = const.tile([128, 1], F32)
        nc.vector.memset(eps_t, EPS)

        qkv = octx.enter_context(tc.tile_pool(name="qkv", bufs=1))
        qT_all = qkv.tile([128, HPC, S], BF)         # [hd, h, tok]
        kT_all = qkv.tile([128, S], BF)              # [hd, tok]
        v_all = qkv.tile([128, SB, HD], BF)          # [tok(P), kb, hd]

        # ---------------- Phase 1: QKV projections + norm/rope -------------
        H5 = HPC + 1
        with tc.tile_pool(name="ph1x", bufs=3) as ph1x, \
             tc.tile_pool(name="ph1", bufs=2) as ph1, \
             tc.tile_pool(name="qps", bufs=2, space="PSUM") as qps_pool, \
             tc.tile_pool(name="kvps", bufs=2, space="PSUM") as kvps_pool, \
             tc.tile_pool(name="trps", bufs=2, space="PSUM") as trps:

            x_tiles = {}

            def load_x(tb):
                ts = slice(tb * 128, (tb + 1) * 128)
                x_sb = ph1x.tile([128, DB, 128], BF, tag="x")
                nc.sync.dma_start(out=x_sb, in_=xh[:, :, ts])
                x_tiles[tb] = x_sb

            def proj(tb):
                q_ps = qps_pool.tile([128, HPC * HD], F32, tag="q_ps")
                kv_ps = kvps_pool.tile([128, 2 * HD], F32, tag="kv_ps")
                x_sb = x_tiles.pop(tb)
                # q and kv interleaved per kb: consecutive matmuls share the
                # same stationary x chunk.
                for kb in range(DB):
                    nc.tensor.matmul(q_ps, lhsT=x_sb[:, kb, :],
                                     rhs=wq_sbs[kb // 4][:, kb % 4, :],
                                     start=(kb == 0), stop=(kb == DB - 1))
                    nc.tensor.matmul(kv_ps, lhsT=x_sb[:, kb, :],
                                     rhs=wkv_sbs[kb // 8][:, kb % 8, :],
                                     start=(kb == 0), stop=(kb == DB - 1))
                return q_ps, kv_ps

            def normrope(tb, q_ps, kv_ps):
                """Act copies out of PSUM, then rms stats + rope + scale on
                DVE over all 5 heads (4q + 1k) in wide fused ops."""
                qk = ph1.tile([128, H5 * HD], BF, tag="qk")
                nc.scalar.copy(out=qk[:, 0:HPC * HD], in_=q_ps)
                nc.scalar.copy(out=qk[:, HPC * HD:], in_=kv_ps[:, 0:HD])
                nc.scalar.copy(out=v_all[:, tb, :], in_=kv_ps[:, HD:])

                qsq = ph1.tile([128, H5 * HD], BF, tag="qsq")
                nc.vector.tensor_mul(qsq, qk, qk)
                rq = ph1.tile([128, H5], F32, tag="rq")
                nc.vector.reduce_sum(
                    rq, qsq.rearrange("p (h c) -> p h c", h=H5),
                    axis=mybir.AxisListType.X)
                nc.scalar.activation(out=rq, in_=rq, func=AF.Sqrt,
                                     bias=eps_t, scale=1.0 / HD)
                nc.vector.reciprocal(out=rq, in_=rq)

                h = HD // 2
                qk5 = qk.rearrange("p (hh c) -> p hh c", hh=H5)
                q4 = qk.rearrange("p (hh c) -> p hh c", hh=H5)[:, 0:HPC, :]
                k1 = qk[:, HPC * HD:]

                t1 = ph1.tile([128, H5 * HD], BF, tag="t1")
                t1v = t1.rearrange("p (hh c) -> p hh c", hh=H5)
                nc.vector.tensor_mul(
                    t1v[:, 0:HPC, :], q4,
                    cq_sb[:, tb, :].unsqueeze(1).broadcast_to([128, HPC, HD]))
                nc.vector.tensor_mul(t1[:, HPC * HD:], k1, ck_sb[:, tb, :])

                t2 = ph1.tile([128, H5 * HD], BF, tag="t2")
                t2v = t2.rearrange("p (hh c) -> p hh c", hh=H5)
                # lo half: rot(x)[c] = -x[c+h]; hi half: rot(x)[c] = x[c-h]
                nc.vector.tensor_mul(
                    t2v[:, 0:HPC, 0:h], q4[:, :, h:HD],
                    sq_sb[:, tb, 0:h].unsqueeze(1).broadcast_to([128, HPC, h]))
                nc.vector.tensor_mul(
                    t2v[:, 0:HPC, h:HD], q4[:, :, 0:h],
                    sq_sb[:, tb, h:HD].unsqueeze(1).broadcast_to([128, HPC, h]))
                nc.vector.tensor_mul(t2v[:, HPC, 0:h], k1[:, h:HD],
                                     sk_sb[:, tb, 0:h])
                nc.vector.tensor_mul(t2v[:, HPC, h:HD], k1[:, 0:h],
                                     sk_sb[:, tb, h:HD])

                ro = ph1.tile([128, H5 * HD], BF, tag="ro")
                rov = ro.rearrange("p (hh c) -> p hh c", hh=H5)
                nc.vector.tensor_sub(rov[:, :, 0:h], t1v[:, :, 0:h],
                                     t2v[:, :, 0:h])
                nc.vector.tensor_add(rov[:, :, h:HD], t1v[:, :, h:HD],
                                     t2v[:, :, h:HD])

                rs = ph1.tile([128, H5 * HD], BF, tag="rs", bufs=3)
                nc.vector.tensor_mul(
                    rs.rearrange("p (hh c) -> p hh c", hh=H5), rov,
                    rq.unsqueeze(2).broadcast_to([128, H5, HD]))
                return rs

            def transposes(tb, rs):
                ts = slice(tb * 128, (tb + 1) * 128)
                for j in range(H5):
                    tr_ps = trps.tile([128, 128], BF, tag="tr")
                    nc.tensor.transpose(tr_ps, rs[:, j * HD:(j + 1) * HD],
                                        ident)
                    if j < HPC:
                        nc.vector.tensor_copy(out=qT_all[:, j, ts], in_=tr_ps)
                    else:
                        nc.vector.tensor_copy(out=kT_all[:, ts], in_=tr_ps)

            nc.sync.dma_start(out=cq_sb, in_=cosq[:, :, :])
            nc.sync.dma_start(out=sq_sb, in_=sinq[:, :, :])
            nc.sync.dma_start(out=ck_sb, in_=cosk[:, :, :])
            nc.sync.dma_start(out=sk_sb, in_=sink[:, :, :])
            load_x(0)
            load_x(1)
            pending = []            # (tb, rs) awaiting transposes, 2 deep
            for tb in range(SB):
                if tb + 2 < SB:
                    load_x(tb + 2)
                q_ps, kv_ps = proj(tb)
                if len(pending) == 2:
                    transposes(*pending.pop(0))
                rs = normrope(tb, q_ps, kv_ps)
                pending.append((tb, rs))
            for pe in pending:
                transposes(*pe)

        # -------- Phase 2+3: attention (unit-pipelined) fused with WO ------
        # Group order ends on the smallest group so the drain tail is short.
        units = [(g, h) for g in (1, 2, 3, 0) for h in range(HPC)]

        with tc.tile_pool(name="expp", bufs=3) as expp, \
             tc.tile_pool(name="attp", bufs=2) as attp, \
             tc.tile_pool(name="ph2", bufs=2) as ph2, \
             tc.tile_pool(name="ysb", bufs=2) as ysb, \
             tc.tile_pool(name="s_ps", bufs=1, space="PSUM") as s_ps_pool, \
             tc.tile_pool(name="rs_ps", bufs=2, space="PSUM") as rs_ps_pool, \
             tc.tile_pool(name="av_ps", bufs=2, space="PSUM") as av_ps_pool, \
             tc.tile_pool(name="y_ps", bufs=2, space="PSUM") as y_ps_pool:

            # One persistent 2-slot score tile (2 PSUM banks); a key block
            # pair (kb0, kb0+1) lands in adjacent banks so one exp can span
            # both.  Subtile dep tracking pipelines writes vs reads.
            s2 = s_ps_pool.tile([128, 2, 512], F32)

            def s2slot(kb):
                return kb % 2

            expT = {}        # u -> tile
            attnT = {}       # g -> tile
            avst = {}        # u -> (rs_ps, av_ps)

            def emit_scores(u):
                """Returns a list of emitter closures, one per key-block
                pair (two score matmuls + one fused exp when the masked
                extents match)."""
                g, h = units[u]
                geb = eblks[g * 4:g * 4 + 4]
                gmax = geb[3]
                eT = expp.tile([128, SB, 512], BF, tag="expT")
                expT[u] = eT

                def jm_of(kb):
                    return sum(1 for e in geb if e <= kb)

                def mm(kb):
                    off = jm_of(kb) * 128
                    nc.tensor.matmul(
                        s2[:, s2slot(kb), off:512],
                        lhsT=kT_all[:, kb * 128:(kb + 1) * 128],
                        rhs=qT_all[:, h, g * 512 + off:(g + 1) * 512],
                        start=True, stop=True)

                def ex(kb0, n):
                    off = jm_of(kb0) * 128
                    sl = s2slot(kb0)
                    nc.scalar.activation(
                        out=eT[:, kb0:kb0 + n, off:512],
                        in_=s2[:, sl:sl + n, off:512],
                        func=AF.Exp, scale=SOFT_SCALE)
                    for kb in range(kb0, kb0 + n):
                        j = kb - g * 4
                        if 0 <= j < 4 and kb * 128 >= p and geb[j] == kb + 1:
                            dsl = eT[:, kb, j * 128:(j + 1) * 128]
                            nc.vector.tensor_mul(dsl, dsl, tri)

                out = []
                for kb0 in range(0, gmax, 2):
                    if kb0 + 1 < gmax:
                        def step(kb0=kb0):
                            mm(kb0)
                            mm(kb0 + 1)
                            if jm_of(kb0) == jm_of(kb0 + 1):
                                ex(kb0, 2)
                            else:
                                ex(kb0, 1)
                                ex(kb0 + 1, 1)
                    else:
                        def step(kb0=kb0):
                            mm(kb0)
                            ex(kb0, 1)
                    out.append(step)
                return out

            def emit_consume(u):
                """Row-sum + AV matmul emitters (one per key block) followed
                by a finalizer emitter (reciprocal + normalize on DVE)."""
                g, h = units[u]
                geb = eblks[g * 4:g * 4 + 4]
                gmax = geb[3]
                eT = expT[u]
                if h == 0:
                    attnT[g] = attp.tile([128, HPC, 512], BF, tag="attnT",
                                         name="attnT")
                out = []
                for kb in range(gmax):
                    def step(kb=kb, geb=geb, gmax=gmax, eT=eT, u=u):
                        jm = sum(1 for e in geb if e <= kb)
                        off = jm * 128
                        if kb == 0:
                            rs_ps = rs_ps_pool.tile([128, 512], F32, tag="rs")
                            av_ps = av_ps_pool.tile([128, 512], F32, tag="av")
                            avst[u] = (rs_ps, av_ps)
                        rs_ps, av_ps = avst[u]
                        nc.tensor.matmul(rs_ps[:, off:512], lhsT=ones_m,
                                         rhs=eT[:, kb, off:512],
                                         start=(kb == 0), stop=(kb == gmax - 1),
                                         skip_group_check=True)
                        nc.tensor.matmul(av_ps[:, off:512],
                                         lhsT=v_all[:, kb, :],
                                         rhs=eT[:, kb, off:512],
                                         start=(kb == 0), stop=(kb == gmax - 1),
                                         skip_group_check=True)
                    out.append(step)

                def fin(u=u, g=g, h=h):
                    rs_ps, av_ps = avst.pop(u)
                    rcp = ph2.tile([128, 512], F32, tag="rcp")
                    nc.vector.reciprocal(out=rcp, in_=rs_ps)
                    nc.vector.tensor_mul(attnT[g][:, h, :], av_ps, rcp)
                    del expT[u]
                out.append(fin)
                return out

            def emit_wo(g):
                aT = attnT.pop(g)
                for db in range(DB):
                    y_ps = y_ps_pool.tile([128, 512], F32, tag="y")
                    for hb in range(HPC):
                        nc.tensor.matmul(
                            y_ps,
                            lhsT=wo_sb[:, hb, db * 128:(db + 1) * 128],
                            rhs=aT[:, hb, :],
                            start=(hb == 0), stop=(hb == HPC - 1))
                    y_sb = ysb.tile([128, 512], BF, tag=f"y_{db % 2}")
                    nc.scalar.copy(out=y_sb, in_=y_ps)
                    nc.sync.dma_start(
                        out=yT[db * 128:(db + 1) * 128,
                               g * 512:(g + 1) * 512],
                        in_=y_sb)

            def interleave(a, b):
                """Merge emitter lists proportionally (a-entries lead)."""
                na, nb = len(a), len(b)
                ia = ib = 0
                while ia < na or ib < nb:
                    if ia < na and (ib >= nb or ia * nb <= ib * na):
                        a[ia]()
                        ia += 1
                    else:
                        b[ib]()
                        ib += 1

            prev = None
            for u in range(len(units)):
                sc = emit_scores(u)
                cons = emit_consume(prev) if prev is not None else []
                interleave(sc, cons)
                g, h = units[u]
                if h == 1 and u >= HPC + 1:
                    # One unit later than the group boundary: gives the h3
                    # normalize (reciprocal on DVE) time to finish before
                    # WO's first PSUM accumulation needs attnT[h3].
                    emit_wo(units[u - 2][0])
                prev = u
            for f in emit_consume(prev):
                f()
            emit_wo(units[-1][0])

    if legalize:
        _legalize_waits(nc)
    return nc


def _to_bf16_tiled(arr2d, inner):
    """[K*128, N] fp32 -> [128, K, N] bf16 (partition-major tiling)."""
    import ml_dtypes
    k = arr2d.shape[0] // 128
    return np.ascontiguousarray(
        arr2d.reshape(k, 128, inner).transpose(1, 0, 2)
    ).astype(ml_dtypes.bfloat16)


def _prep_inputs(x, cos, sin, wq, wk, wv, wo, q_gamma, k_gamma, p):
    """Build the 8 per-core input maps (all bf16, pre-tiled)."""
    import ml_dtypes
    cos2 = np.asarray(cos, np.float32).reshape(S, HD)
    sin2 = np.asarray(sin, np.float32).reshape(S, HD)
    qg = np.asarray(q_gamma, np.float32)
    kg = np.asarray(k_gamma, np.float32)
    h = HD // 2
    qg_rot = np.concatenate([qg[h:], qg[:h]])
    kg_rot = np.concatenate([kg[h:], kg[:h]])
    cos_q = _to_bf16_tiled(cos2 * qg, HD)
    sin_q = _to_bf16_tiled(sin2 * qg_rot, HD)
    cos_k = _to_bf16_tiled(cos2 * kg, HD)
    sin_k = _to_bf16_tiled(sin2 * kg_rot, HD)

    ii = np.arange(128)
    tri = (ii[:, None] <= ii[None, :]).astype(ml_dtypes.bfloat16)
    ident = np.eye(128, dtype=ml_dtypes.bfloat16)
    ones = np.ones((128, 128), dtype=ml_dtypes.bfloat16)

    x = np.asarray(x, np.float32)
    wq = np.asarray(wq, np.float32)
    wk = np.asarray(wk, np.float32)
    wv = np.asarray(wv, np.float32)
    wo = np.asarray(wo, np.float32)

    xh = [_to_bf16_tiled(np.ascontiguousarray(x[b].T), S) for b in range(B)]
    in_maps = []
    for c in range(N_CORES):
        b, g = divmod(c, N_CORES // B)
        h0 = g * HPC
        kv = h0 // (NH // KVH)
        wqc = _to_bf16_tiled(
            np.ascontiguousarray(wq[h0 * HD:(h0 + HPC) * HD, :].T), HPC * HD)
        wkvc = _to_bf16_tiled(np.ascontiguousarray(
            np.concatenate([wk[kv * HD:(kv + 1) * HD, :],
                            wv[kv * HD:(kv + 1) * HD, :]], axis=0).T), 2 * HD)
        woc = _to_bf16_tiled(
            np.ascontiguousarray(wo[:, h0 * HD:(h0 + HPC) * HD].T), D)
        in_maps.append({
            "xh": xh[b], "wq": wqc, "wkv": wkvc, "wo": woc,
            "cosq": cos_q, "sinq": sin_q, "cosk": cos_k, "sink": sin_k,
            "identd": ident, "trid": tri, "onesd": ones,
        })
    return in_maps


def _gather(results):
    y = np.zeros((B, S, D), dtype=np.float32)
    for c in range(N_CORES):
        b = c // (N_CORES // B)
        y[b] += results[c]["yT"].astype(np.float32).T
    return y


def kernel(x, cos, sin, wq, wk, wv, wo, q_gamma, k_gamma, signal_token_num):
    p = int(signal_token_num)
    assert p % 128 == 0 and 0 <= p <= S, f"unsupported signal_token_num {p}"

    nc = build_core_kernel(p)
    in_maps = _prep_inputs(x, cos, sin, wq, wk, wv, wo, q_gamma, k_gamma, p)
    res = run_bass_kernel_spmd(nc, in_maps, list(range(N_CORES)))
    return _gather(res.results)


def _install_ntff_hook():
    """The container's antenv lacks axon_hooks; replicate the boot-time NTFF
    profile hook (ctypes into libaxon_pjrt.so) and register the module."""
    import sys
    import types
    import ctypes
    import contextlib

    if "antenv.axon_hooks" in sys.modules:
        return
    so_path = "/opt/axon/libaxon_pjrt.so"
    lib = ctypes.CDLL(so_path)
    if not hasattr(lib, "axon_start_nrt_profile"):
        return
    lib.axon_start_nrt_profile.argtypes = [
        ctypes.POINTER(ctypes.c_int64), ctypes.c_size_t]
    lib.axon_start_nrt_profile.restype = ctypes.c_int64
    lib.axon_stop_nrt_profile.argtypes = [ctypes.c_char_p]
    lib.axon_stop_nrt_profile.restype = ctypes.c_int64

    @contextlib.contextmanager
    def _hook(output_dir, device_ids):
        import jax
        jax.devices()
        if device_ids:
            ids = (ctypes.c_int64 * len(device_ids))(*device_ids)
            rc = lib.axon_start_nrt_profile(ids, len(device_ids))
        else:
            rc = lib.axon_start_nrt_profile(None, 0)
        if rc != 0:
            raise RuntimeError(f"axon_start_nrt_profile rc={rc}")
        try:
            yield
        finally:
            n = lib.axon_stop_nrt_profile(str(output_dir).encode())
            print(f"profile: {n} file(s) written to {output_dir}")

    import antenv
    mod = types.ModuleType("antenv.axon_hooks")
    mod.get_axon_ntff_profile_hook = lambda: _hook
    mod.set_axon_ntff_profile_hook = lambda h: None
    sys.modules["antenv.axon_hooks"] = mod
    antenv.axon_hooks = mod


def profile_once(inputs):
    """Run once with NTFF tracing; return max per-core exec time in ns."""
    import concourse.bass_utils as bu
    bu.upload_artifacts = lambda tmpdir: ""   # no bucket access here
    _install_ntff_hook()
    p = int(inputs["signal_token_num"])
    nc = build_core_kernel(p)
    in_maps = _prep_inputs(
        inputs["x"], inputs["cos"], inputs["sin"], inputs["wq"], inputs["wk"],
        inputs["wv"], inputs["wo"], inputs["q_gamma"], inputs["k_gamma"], p)
    try:
        res = bu.run_bass_kernel_spmd(nc, in_maps, list(range(N_CORES)),
                                      trace=True,
                                      trace_cores=list(range(N_CORES)))
        return res.exec_time_ns
    except Exception as e:
        print(f"profile failed: {type(e).__name__}: {e}")
        return None


# revision 25
# speedup vs baseline: 1.1810x; 1.1810x over previous
"""Trainium2 Bass kernel for GQA attention with QK-RMSNorm, RoPE and a
bidirectional-prefix + causal mask (sparse_attention problem).

Reference computation (fp32):
  xq = x @ wq.T; xk = x @ wk.T; xv = x @ wv.T   (per-head RMSNorm on q,k)
  rope(q), rope(k); repeat kv heads 8x
  scores = q k^T / sqrt(128); mask = causal OR (i<p & j<p)
  out = softmax(scores) @ v;  y = out @ wo.T

Sharding: 8 cores = 2 batches x 4 head-groups (4 query heads each, sharing
one KV head).  Each core computes a partial y^T (its 4 heads' contribution);
the host sums the 4 partials per batch and transposes back.

All matmul operands are bf16 (fp32 PSUM accumulation); host pre-converts and
pre-tiles every input so DMAs land in their final SBUF layout with no
on-device dtype staging.  Row sums use an all-ones [128,128] stationary so
the softmax denominator lands pre-broadcast across partitions.  Scores /
row-sums / AV matmuls are narrowed to the exact masked extent per key block;
the causal diagonal is applied post-exp with a 0/1 triangular mask multiply.
Emission interleaves scores(u) with consume(u-1) and WO so the PE never
stalls on the scalar engine's exp.

TRN2 ISA allows ONE sync-wait per instruction and walrus does not split
multi-wait instructions, so `_legalize_waits` rewrites the emitted BIR,
moving excess waits onto preceding same-engine NoOps.
"""
import math
import numpy as np
from contextlib import ExitStack

import bass_rust
import concourse.bass as bass
import concourse.mybir as mybir
import concourse.tile as tile
from concourse.bass_utils import run_bass_kernel_spmd

F32 = mybir.dt.float32
BF = mybir.dt.bfloat16
AF = mybir.ActivationFunctionType

B, S, D = 2, 2048, 2048
NH, KVH, HD = 16, 2, 128
HPC = 4                      # query heads per core
N_CORES = 8
EPS = 1e-6
SOFT_SCALE = 1.0 / math.sqrt(HD)

SB = S // 128                # 16 token blocks
DB = D // 128                # 16 contraction blocks
NG = SB // 4                 # 4 query groups of 512 tokens

_lgw_counter = [0]


def _legalize_waits(nc, cap=1):
    """Move all-but-`cap` sync waits of every instruction onto preceding
    same-engine NoOps (TRN2 EVENTS block has a single wait slot)."""
    for fn in nc.m.functions:
        for blk in fn.blocks:
            out = []
            changed = False
            for inst in blk.instructions:
                si = inst.sync_info
                waits = list(si.on_wait) if si is not None and si.on_wait else []
                if len(waits) > cap:
                    changed = True
                    move, keep = waits[:-cap], waits[-cap:]
                    for w in move:
                        n = bass_rust.InstNoOp(name=f"LGW-{_lgw_counter[0]}")
                        _lgw_counter[0] += 1
                        n.engine = inst.engine
                        n.sync_info = mybir.SyncInfo(on_wait=[w], on_update=[])
                        out.append(n)
                    inst.sync_info = mybir.SyncInfo(
                        on_wait=keep, on_update=list(si.on_update or []))
                out.append(inst)
            if changed:
                blk.instructions = out
    return nc


def _eblk(rb, p):
    """Key extent (in 128-blocks) attended by query row-block rb."""
    hi = (rb + 1) * 128
    return (p if hi <= p else hi) // 128


def build_core_kernel(p, legalize=True):
    """One SPMD program; per-core behavior differs only via input data."""
    nc = bass.Bass()

    xh = nc.dram_tensor("xh", [128, DB, S], BF, kind="ExternalInput")
    wq = nc.dram_tensor("wq", [128, DB, HPC * HD], BF, kind="ExternalInput")
    wkv = nc.dram_tensor("wkv", [128, DB, 2 * HD], BF, kind="ExternalInput")
    wo = nc.dram_tensor("wo", [128, HPC, D], BF, kind="ExternalInput")
    cosq = nc.dram_tensor("cosq", [128, SB, HD], BF, kind="ExternalInput")
    sinq = nc.dram_tensor("sinq", [128, SB, HD], BF, kind="ExternalInput")
    cosk = nc.dram_tensor("cosk", [128, SB, HD], BF, kind="ExternalInput")
    sink = nc.dram_tensor("sink", [128, SB, HD], BF, kind="ExternalInput")
    identd = nc.dram_tensor("identd", [128, 128], BF, kind="ExternalInput")
    trid = nc.dram_tensor("trid", [128, 128], BF, kind="ExternalInput")
    onesd = nc.dram_tensor("onesd", [128, 128], BF, kind="ExternalInput")
    yT = nc.dram_tensor("yT", [D, S], BF, kind="ExternalOutput")

    eblks = [_eblk(rb, p) for rb in range(SB)]

    with tile.TileContext(nc) as tc, ExitStack() as octx:
        # Const/weight loads go on the Activation HWDGE queue so they overlap
        # the x-tile loads on the SP queue; ordered by first use (wq -> wkv
        # -> rope tables -> identity; wo/tri/ones only matter in phase 2).
        const = octx.enter_context(tc.tile_pool(name="const", bufs=1))
        # Separate tiles per weight chunk -> the first projection matmul only
        # waits on the first 4-block chunk, not the whole weight load.
        wq_sbs = []
        for c4 in range(4):
            wq_c = const.tile([128, 4, HPC * HD], BF, name=f"wq_c{c4}")
            nc.scalar.dma_start(out=wq_c, in_=wq[:, c4 * 4:(c4 + 1) * 4, :])
            wq_sbs.append(wq_c)
        wkv_sbs = []
        for c2 in range(2):
            wkv_c = const.tile([128, 8, 2 * HD], BF, name=f"wkv_c{c2}")
            nc.scalar.dma_start(out=wkv_c, in_=wkv[:, c2 * 8:(c2 + 1) * 8, :])
            wkv_sbs.append(wkv_c)
        # Rope tables ride the SP queue (behind x0/x1, ahead of x2+) so the
        # Act queue reaches wq/wkv sooner.
        cq_sb = const.tile([128, SB, HD], BF)
        sq_sb = const.tile([128, SB, HD], BF)
        ck_sb = const.tile([128, SB, HD], BF)
        sk_sb = const.tile([128, SB, HD], BF)
        ident = const.tile([128, 128], BF)
        nc.scalar.dma_start(out=ident, in_=identd[:, :])
        wo_sb = const.tile([128, HPC, D], BF)
        nc.scalar.dma_start(out=wo_sb, in_=wo[:, :, :])
        tri = const.tile([128, 128], BF)
        nc.scalar.dma_start(out=tri, in_=trid[:, :])
        ones_m = const.tile([128, 128], BF)
        nc.scalar.dma_start(out=ones_m, in_=onesd[:, :])
        eps_t = const.tile([128, 1], F32)
        nc.vector.memset(eps_t, EPS)

        qkv = octx.enter_context(tc.tile_pool(name="qkv", bufs=1))
        qT_all = qkv.tile([128, HPC, S], BF)         # [hd, h, tok]
        kT_all = qkv.tile([128, S], BF)              # [hd, tok]
        v_all = qkv.tile([128, SB, HD], BF)          # [tok(P), kb, hd]

        # ---------------- Phase 1: QKV projections + norm/rope -------------
        H5 = HPC + 1
        with tc.tile_pool(name="ph1x", bufs=3) as ph1x, \
             tc.tile_pool(name="ph1", bufs=2) as ph1, \
             tc.tile_pool(name="qps", bufs=2, space="PSUM") as qps_pool, \
             tc.tile_pool(name="kvps", bufs=2, space="PSUM") as kvps_pool, \
             tc.tile_pool(name="trps", bufs=2, space="PSUM") as trps:

            x_tiles = {}

            def load_x(tb):
                ts = slice(tb * 128, (tb + 1) * 128)
                x_sb = ph1x.tile([128, DB, 128], BF, tag="x")
                nc.sync.dma_start(out=x_sb, in_=xh[:, :, ts])
                x_tiles[tb] = x_sb

            def proj(tb):
                q_ps = qps_pool.tile([128, HPC * HD], F32, tag="q_ps")
                kv_ps = kvps_pool.tile([128, 2 * HD], F32, tag="kv_ps")
                x_sb = x_tiles.pop(tb)
                for kb in range(DB):
                    nc.tensor.matmul(q_ps, lhsT=x_sb[:, kb, :],
                                     rhs=wq_sbs[kb // 4][:, kb % 4, :],
                                     start=(kb == 0), stop=(kb == DB - 1))
                for kb in range(DB):
                    nc.tensor.matmul(kv_ps, lhsT=x_sb[:, kb, :],
                                     rhs=wkv_sbs[kb // 8][:, kb % 8, :],
                                     start=(kb == 0), stop=(kb == DB - 1))
                return q_ps, kv_ps

            def normrope(tb, q_ps, kv_ps):
                """Act copies out of PSUM, then rms stats + rope + scale on
                DVE over all 5 heads (4q + 1k) in wide fused ops."""
                qk = ph1.tile([128, H5 * HD], BF, tag="qk")
                nc.scalar.copy(out=qk[:, 0:HPC * HD], in_=q_ps)
                nc.scalar.copy(out=qk[:, HPC * HD:], in_=kv_ps[:, 0:HD])
                nc.scalar.copy(out=v_all[:, tb, :], in_=kv_ps[:, HD:])

                qsq = ph1.tile([128, H5 * HD], BF, tag="qsq")
                nc.vector.tensor_mul(qsq, qk, qk)
                rq = ph1.tile([128, H5], F32, tag="rq")
                nc.vector.reduce_sum(
                    rq, qsq.rearrange("p (h c) -> p h c", h=H5),
                    axis=mybir.AxisListType.X)
                nc.scalar.activation(out=rq, in_=rq, func=AF.Sqrt,
                                     bias=eps_t, scale=1.0 / HD)
                nc.vector.reciprocal(out=rq, in_=rq)

                h = HD // 2
                qk5 = qk.rearrange("p (hh c) -> p hh c", hh=H5)
                q4 = qk.rearrange("p (hh c) -> p hh c", hh=H5)[:, 0:HPC, :]
                k1 = qk[:, HPC * HD:]

                t1 = ph1.tile([128, H5 * HD], BF, tag="t1")
                t1v = t1.rearrange("p (hh c) -> p hh c", hh=H5)
                nc.vector.tensor_mul(
                    t1v[:, 0:HPC, :], q4,
                    cq_sb[:, tb, :].unsqueeze(1).broadcast_to([128, HPC, HD]))
                nc.vector.tensor_mul(t1[:, HPC * HD:], k1, ck_sb[:, tb, :])

                t2 = ph1.tile([128, H5 * HD], BF, tag="t2")
                t2v = t2.rearrange("p (hh c) -> p hh c", hh=H5)
                # lo half: rot(x)[c] = -x[c+h]; hi half: rot(x)[c] = x[c-h]
                nc.vector.tensor_mul(
                    t2v[:, 0:HPC, 0:h], q4[:, :, h:HD],
                    sq_sb[:, tb, 0:h].unsqueeze(1).broadcast_to([128, HPC, h]))
                nc.vector.tensor_mul(
                    t2v[:, 0:HPC, h:HD], q4[:, :, 0:h],
                    sq_sb[:, tb, h:HD].unsqueeze(1).broadcast_to([128, HPC, h]))
                nc.vector.tensor_mul(t2v[:, HPC, 0:h], k1[:, h:HD],
                                     sk_sb[:, tb, 0:h])
                nc.vector.tensor_mul(t2v[:, HPC, h:HD], k1[:, 0:h],
                                     sk_sb[:, tb, h:HD])

                ro = ph1.tile([128, H5 * HD], BF, tag="ro")
                rov = ro.rearrange("p (hh c) -> p hh c", hh=H5)
                nc.vector.tensor_sub(rov[:, :, 0:h], t1v[:, :, 0:h],
                                     t2v[:, :, 0:h])
                nc.vector.tensor_add(rov[:, :, h:HD], t1v[:, :, h:HD],
                                     t2v[:, :, h:HD])

                rs = ph1.tile([128, H5 * HD], BF, tag="rs", bufs=3)
                nc.vector.tensor_mul(
                    rs.rearrange("p (hh c) -> p hh c", hh=H5), rov,
                    rq.unsqueeze(2).broadcast_to([128, H5, HD]))
                return rs

            def transposes(tb, rs):
                ts = slice(tb * 128, (tb + 1) * 128)
                for j in range(H5):
                    tr_ps = trps.tile([128, 128], BF, tag="tr")
                    nc.tensor.transpose(tr_ps, rs[:, j * HD:(j + 1) * HD],
                                        ident)
                    if j < HPC:
                        nc.vector.tensor_copy(out=qT_all[:, j, ts], in_=tr_ps)
                    else:
                        nc.vector.tensor_copy(out=kT_all[:, ts], in_=tr_ps)

            nc.sync.dma_start(out=cq_sb, in_=cosq[:, :, :])
            nc.sync.dma_start(out=sq_sb, in_=sinq[:, :, :])
            nc.sync.dma_start(out=ck_sb, in_=cosk[:, :, :])
            nc.sync.dma_start(out=sk_sb, in_=sink[:, :, :])
            load_x(0)
            load_x(1)
            pending = []            # (tb, rs) awaiting transposes, 2 deep
            for tb in range(SB):
                if tb + 2 < SB:
                    load_x(tb + 2)
                q_ps, kv_ps = proj(tb)
                if len(pending) == 2:
                    transposes(*pending.pop(0))
                rs = normrope(tb, q_ps, kv_ps)
                pending.append((tb, rs))
            for pe in pending:
                transposes(*pe)

        # -------- Phase 2+3: attention (unit-pipelined) fused with WO ------
        # Group order ends on the smallest group so the drain tail is short.
        units = [(g, h) for g in (1, 2, 3, 0) for h in range(HPC)]

        with tc.tile_pool(name="expp", bufs=3) as expp, \
             tc.tile_pool(name="attp", bufs=2) as attp, \
             tc.tile_pool(name="ph2", bufs=2) as ph2, \
             tc.tile_pool(name="ysb", bufs=2) as ysb, \
             tc.tile_pool(name="s_ps", bufs=1, space="PSUM") as s_ps_pool, \
             tc.tile_pool(name="rs_ps", bufs=2, space="PSUM") as rs_ps_pool, \
             tc.tile_pool(name="av_ps", bufs=2, space="PSUM") as av_ps_pool, \
             tc.tile_pool(name="y_ps", bufs=2, space="PSUM") as y_ps_pool:

            # One persistent 2-slot score tile (2 PSUM banks); a key block
            # pair (kb0, kb0+1) lands in adjacent banks so one exp can span
            # both.  Subtile dep tracking pipelines writes vs reads.
            s2 = s_ps_pool.tile([128, 2, 512], F32)

            def s2slot(kb):
                return kb % 2

            expT = {}        # u -> tile
            attnT = {}       # g -> tile
            avst = {}        # u -> (rs_ps, av_ps)

            def emit_scores(u):
                """Returns a list of emitter closures, one per key-block
                pair (two score matmuls + one fused exp when the masked
                extents match)."""
                g, h = units[u]
                geb = eblks[g * 4:g * 4 + 4]
                gmax = geb[3]
                eT = expp.tile([128, SB, 512], BF, tag="expT")
                expT[u] = eT

                def jm_of(kb):
                    return sum(1 for e in geb if e <= kb)

                def mm(kb):
                    off = jm_of(kb) * 128
                    nc.tensor.matmul(
                        s2[:, s2slot(kb), off:512],
                        lhsT=kT_all[:, kb * 128:(kb + 1) * 128],
                        rhs=qT_all[:, h, g * 512 + off:(g + 1) * 512],
                        start=True, stop=True)

                def ex(kb0, n):
                    off = jm_of(kb0) * 128
                    sl = s2slot(kb0)
                    nc.scalar.activation(
                        out=eT[:, kb0:kb0 + n, off:512],
                        in_=s2[:, sl:sl + n, off:512],
                        func=AF.Exp, scale=SOFT_SCALE)
                    for kb in range(kb0, kb0 + n):
                        j = kb - g * 4
                        if 0 <= j < 4 and kb * 128 >= p and geb[j] == kb + 1:
                            dsl = eT[:, kb, j * 128:(j + 1) * 128]
                            nc.vector.tensor_mul(dsl, dsl, tri)

                out = []
                for kb0 in range(0, gmax, 2):
                    if kb0 + 1 < gmax:
                        def step(kb0=kb0):
                            mm(kb0)
                            mm(kb0 + 1)
                            if jm_of(kb0) == jm_of(kb0 + 1):
                                ex(kb0, 2)
                            else:
                                ex(kb0, 1)
                                ex(kb0 + 1, 1)
                    else:
                        def step(kb0=kb0):
                            mm(kb0)
                            ex(kb0, 1)
                    out.append(step)
                return out

            def emit_consume(u):
                """Row-sum + AV matmul emitters (one per key block) followed
                by a finalizer emitter (reciprocal + normalize on DVE)."""
                g, h = units[u]
                geb = eblks[g * 4:g * 4 + 4]
                gmax = geb[3]
                eT = expT[u]
                if h == 0:
                    attnT[g] = attp.tile([128, HPC, 512], BF, tag="attnT",
                                         name="attnT")
                out = []
                for kb in range(gmax):
                    def step(kb=kb, geb=geb, gmax=gmax, eT=eT, u=u):
                        jm = sum(1 for e in geb if e <= kb)
                        off = jm * 128
                        if kb == 0:
                            rs_ps = rs_ps_pool.tile([128, 512], F32, tag="rs")
                            av_ps = av_ps_pool.tile([128, 512], F32, tag="av")
                            avst[u] = (rs_ps, av_ps)
                        rs_ps, av_ps = avst[u]
                        nc.tensor.matmul(rs_ps[:, off:512], lhsT=ones_m,
                                         rhs=eT[:, kb, off:512],
                                         start=(kb == 0), stop=(kb == gmax - 1),
                                         skip_group_check=True)
                        nc.tensor.matmul(av_ps[:, off:512],
                                         lhsT=v_all[:, kb, :],
                                         rhs=eT[:, kb, off:512],
                                         start=(kb == 0), stop=(kb == gmax - 1),
                                         skip_group_check=True)
                    out.append(step)

                def fin(u=u, g=g, h=h):
                    rs_ps, av_ps = avst.pop(u)
                    rcp = ph2.tile([128, 512], F32, tag="rcp")
                    nc.vector.reciprocal(out=rcp, in_=rs_ps)
                    nc.vector.tensor_mul(attnT[g][:, h, :], av_ps, rcp)
                    del expT[u]
                out.append(fin)
                return out

            def emit_wo(g):
                aT = attnT.pop(g)
                for db in range(DB):
                    y_ps = y_ps_pool.tile([128, 512], F32, tag="y")
                    for hb in range(HPC):
                        nc.tensor.matmul(
                            y_ps,
                            lhsT=wo_sb[:, hb, db * 128:(db + 1) * 128],
                            rhs=aT[:, hb, :],
                            start=(hb == 0), stop=(hb == HPC - 1))
                    y_sb = ysb.tile([128, 512], BF, tag=f"y_{db % 2}")
                    nc.scalar.copy(out=y_sb, in_=y_ps)
                    nc.sync.dma_start(
                        out=yT[db * 128:(db + 1) * 128,
                               g * 512:(g + 1) * 512],
                        in_=y_sb)

            def interleave(a, b):
                """Merge emitter lists proportionally (a-entries lead)."""
                na, nb = len(a), len(b)
                ia = ib = 0
                while ia < na or ib < nb:
                    if ia < na and (ib >= nb or ia * nb <= ib * na):
                        a[ia]()
                        ia += 1
                    else:
                        b[ib]()
                        ib += 1

            prev = None
            for u in range(len(units)):
                sc = emit_scores(u)
                cons = emit_consume(prev) if prev is not None else []
                interleave(sc, cons)
                g, h = units[u]
                if h == 1 and u >= HPC + 1:
                    # One unit later than the group boundary: gives the h3
                    # normalize (reciprocal on DVE) time to finish before
                    # WO's first PSUM accumulation needs attnT[h3].
                    emit_wo(units[u - 2][0])
                prev = u
            for f in emit_consume(prev):
                f()
            emit_wo(units[-1][0])

    if legalize:
        _legalize_waits(nc)
    return nc


def _to_bf16_tiled(arr2d, inner):
    """[K*128, N] fp32 -> [128, K, N] bf16 (partition-major tiling)."""
    import ml_dtypes
    k = arr2d.shape[0] // 128
    return np.ascontiguousarray(
        arr2d.reshape(k, 128, inner).transpose(1, 0, 2)
    ).astype(ml_dtypes.bfloat16)


def _prep_inputs(x, cos, sin, wq, wk, wv, wo, q_gamma, k_gamma, p):
    """Build the 8 per-core input maps (all bf16, pre-tiled)."""
    import ml_dtypes
    cos2 = np.asarray(cos, np.float32).reshape(S, HD)
    sin2 = np.asarray(sin, np.float32).reshape(S, HD)
    qg = np.asarray(q_gamma, np.float32)
    kg = np.asarray(k_gamma, np.float32)
    h = HD // 2
    qg_rot = np.concatenate([qg[h:], qg[:h]])
    kg_rot = np.concatenate([kg[h:], kg[:h]])
    cos_q = _to_bf16_tiled(cos2 * qg, HD)
    sin_q = _to_bf16_tiled(sin2 * qg_rot, HD)
    cos_k = _to_bf16_tiled(cos2 * kg, HD)
    sin_k = _to_bf16_tiled(sin2 * kg_rot, HD)

    ii = np.arange(128)
    tri = (ii[:, None] <= ii[None, :]).astype(ml_dtypes.bfloat16)
    ident = np.eye(128, dtype=ml_dtypes.bfloat16)
    ones = np.ones((128, 128), dtype=ml_dtypes.bfloat16)

    x = np.asarray(x, np.float32)
    wq = np.asarray(wq, np.float32)
    wk = np.asarray(wk, np.float32)
    wv = np.asarray(wv, np.float32)
    wo = np.asarray(wo, np.float32)

    xh = [_to_bf16_tiled(np.ascontiguousarray(x[b].T), S) for b in range(B)]
    in_maps = []
    for c in range(N_CORES):
        b, g = divmod(c, N_CORES // B)
        h0 = g * HPC
        kv = h0 // (NH // KVH)
        wqc = _to_bf16_tiled(
            np.ascontiguousarray(wq[h0 * HD:(h0 + HPC) * HD, :].T), HPC * HD)
        wkvc = _to_bf16_tiled(np.ascontiguousarray(
            np.concatenate([wk[kv * HD:(kv + 1) * HD, :],
                            wv[kv * HD:(kv + 1) * HD, :]], axis=0).T), 2 * HD)
        woc = _to_bf16_tiled(
            np.ascontiguousarray(wo[:, h0 * HD:(h0 + HPC) * HD].T), D)
        in_maps.append({
            "xh": xh[b], "wq": wqc, "wkv": wkvc, "wo": woc,
            "cosq": cos_q, "sinq": sin_q, "cosk": cos_k, "sink": sin_k,
            "identd": ident, "trid": tri, "onesd": ones,
        })
    return in_maps


def _gather(results):
    y = np.zeros((B, S, D), dtype=np.float32)
    for c in range(N_CORES):
        b = c // (N_CORES // B)
        y[b] += results[c]["yT"].astype(np.float32).T
    return y


def kernel(x, cos, sin, wq, wk, wv, wo, q_gamma, k_gamma, signal_token_num):
    p = int(signal_token_num)
    assert p % 128 == 0 and 0 <= p <= S, f"unsupported signal_token_num {p}"

    nc = build_core_kernel(p)
    in_maps = _prep_inputs(x, cos, sin, wq, wk, wv, wo, q_gamma, k_gamma, p)
    res = run_bass_kernel_spmd(nc, in_maps, list(range(N_CORES)))
    return _gather(res.results)


def _install_ntff_hook():
    """The container's antenv lacks axon_hooks; replicate the boot-time NTFF
    profile hook (ctypes into libaxon_pjrt.so) and register the module."""
    import sys
    import types
    import ctypes
    import contextlib

    if "antenv.axon_hooks" in sys.modules:
        return
    so_path = "/opt/axon/libaxon_pjrt.so"
    lib = ctypes.CDLL(so_path)
    if not hasattr(lib, "axon_start_nrt_profile"):
        return
    lib.axon_start_nrt_profile.argtypes = [
        ctypes.POINTER(ctypes.c_int64), ctypes.c_size_t]
    lib.axon_start_nrt_profile.restype = ctypes.c_int64
    lib.axon_stop_nrt_profile.argtypes = [ctypes.c_char_p]
    lib.axon_stop_nrt_profile.restype = ctypes.c_int64

    @contextlib.contextmanager
    def _hook(output_dir, device_ids):
        import jax
        jax.devices()
        if device_ids:
            ids = (ctypes.c_int64 * len(device_ids))(*device_ids)
            rc = lib.axon_start_nrt_profile(ids, len(device_ids))
        else:
            rc = lib.axon_start_nrt_profile(None, 0)
        if rc != 0:
            raise RuntimeError(f"axon_start_nrt_profile rc={rc}")
        try:
            yield
        finally:
            n = lib.axon_stop_nrt_profile(str(output_dir).encode())
            print(f"profile: {n} file(s) written to {output_dir}")

    import antenv
    mod = types.ModuleType("antenv.axon_hooks")
    mod.get_axon_ntff_profile_hook = lambda: _hook
    mod.set_axon_ntff_profile_hook = lambda h: None
    sys.modules["antenv.axon_hooks"] = mod
    antenv.axon_hooks = mod


def profile_once(inputs):
    """Run once with NTFF tracing; return max per-core exec time in ns."""
    import concourse.bass_utils as bu
    bu.upload_artifacts = lambda tmpdir: ""   # no bucket access here
    _install_ntff_hook()
    p = int(inputs["signal_token_num"])
    nc = build_core_kernel(p)
    in_maps = _prep_inputs(
        inputs["x"], inputs["cos"], inputs["sin"], inputs["wq"], inputs["wk"],
        inputs["wv"], inputs["wo"], inputs["q_gamma"], inputs["k_gamma"], p)
    try:
        res = bu.run_bass_kernel_spmd(nc, in_maps, list(range(N_CORES)),
                                      trace=True,
                                      trace_cores=list(range(N_CORES)))
        return res.exec_time_ns
    except Exception as e:
        print(f"profile failed: {type(e).__name__}: {e}")
        return None
